# revision 6
# baseline (speedup 1.0000x reference)
"""ConvolutionalDRAW Trainium2 kernel (Bass/Tile, 8 NeuronCores, pure data parallel).

Strategy
--------
- Pure data parallel: 8 images per core (batch 64 / 8 cores). No collectives;
  the kl partial sums are combined on the host during unsharding.
- Each core processes its 8 images in 4 passes of 2 images. Per pass the full
  8-step DRAW recurrence runs with all activations SBUF-resident.
- Convolutions are computed channel-major as shift-and-accumulate matmuls:
  activations live in zero-padded planes [ch, img, 20, 20]; kernel tap (ky,kx)
  contributes one matmul lhsT=W[ci,co], rhs=plane[ci, :, ky:ky+16, kx:kx+16],
  accumulated in PSUM over (ci-chunk, tap). N = 2 img * 256 pix = 512.
- All matmuls run in float32r (full-rate fp32 on trn2 PE, ~19-bit mantissa).
- Weights are streamed from HBM per (conv, ci-chunk, co-chunk) block in a
  host-pretransposed [tap, ci, co] layout, double-buffered.
- The write transpose-conv (stride 4 = kernel 4) is 16 disjoint-tap matmuls
  accumulated into a tap-major u buffer; the 1x1 obs conv + sigmoid runs at
  pass end, assembled into canvas layout on-chip, then DMA'd out contiguously.
"""
import numpy as np

import concourse.bass as bass
import concourse.mybir as mybir
import concourse.tile as tile
from concourse import bacc
from concourse.bass_utils import run_bass_kernel_spmd

F32 = mybir.dt.float32
F32R = mybir.dt.float32r
AF = mybir.ActivationFunctionType
OP = mybir.AluOpType

NCORES = 8
B = 64
BPC = B // NCORES          # images per core
IPP = 2                    # images per pass
NPASS = BPC // IPP
NL = 8                     # DRAW steps
HS, PD = 16, 20            # spatial, padded
NPIX = IPP * HS * HS       # matmul free size (512)
INT = slice(2, 18)         # interior of padded plane

# channel chunking (ci side) of the two recurrent conv inputs
ENC_CI = [(0, 128), (128, 128), (256, 128), (384, 10)]    # enc concat: 394
DEC_CI = [(0, 128), (128, 128), (256, 71)]                # dec concat: 327
CO_G = [(0, 128), (128, 128), (256, 128), (384, 128)]     # gates: 512
CO_ET = [(0, 128), (128, 128), (256, 128), (384, 10)]     # enc transform out: 394
CO_DT = [(0, 128), (128, 128), (256, 71)]                 # dec transform out: 327
CO_1 = [(0, 128)]


def _build():
    nc = bacc.Bacc("TRN2", target_bir_lowering=False, debug=False,
                   num_devices=NCORES)

    # ---- DRAM I/O (per core) ----
    xin = nc.dram_tensor("xin", [BPC, 3, 64, 64], F32, kind="ExternalInput")
    vin = nc.dram_tensor("vin", [BPC, 7], F32, kind="ExternalInput")
    rin = nc.dram_tensor("rin", [BPC, 256, HS, HS], F32, kind="ExternalInput")
    epsin = nc.dram_tensor("epsin", [NL, BPC, 64, HS, HS], F32,
                           kind="ExternalInput")
    # host-pretransposed weights [tap, ci, co]
    w_pri = nc.dram_tensor("w_pri", [25, 128, 128], F32, kind="ExternalInput")
    w_pos = nc.dram_tensor("w_pos", [25, 128, 128], F32, kind="ExternalInput")
    w_et = nc.dram_tensor("w_et", [25, 128, 394], F32, kind="ExternalInput")
    w_eg = nc.dram_tensor("w_eg", [25, 394, 512], F32, kind="ExternalInput")
    w_dt = nc.dram_tensor("w_dt", [25, 128, 327], F32, kind="ExternalInput")
    w_dg = nc.dram_tensor("w_dg", [25, 327, 512], F32, kind="ExternalInput")
    w_wr = nc.dram_tensor("w_wr", [16, 128, 128], F32, kind="ExternalInput")
    w_rd = nc.dram_tensor("w_rd", [16, 3, 3], F32, kind="ExternalInput")
    w_ob = nc.dram_tensor("w_ob", [128, 3], F32, kind="ExternalInput")
    b_pri = nc.dram_tensor("b_pri", [128], F32, kind="ExternalInput")
    b_pos = nc.dram_tensor("b_pos", [128], F32, kind="ExternalInput")
    b_eg = nc.dram_tensor("b_eg", [512], F32, kind="ExternalInput")
    b_et = nc.dram_tensor("b_et", [394], F32, kind="ExternalInput")
    b_dg = nc.dram_tensor("b_dg", [512], F32, kind="ExternalInput")
    b_dt = nc.dram_tensor("b_dt", [327], F32, kind="ExternalInput")
    b_ob = nc.dram_tensor("b_ob", [3], F32, kind="ExternalInput")

    canvas = nc.dram_tensor("canvas", [BPC, 3, 64, 64], F32,
                            kind="ExternalOutput")
    klout = nc.dram_tensor("klout", [1, 1], F32, kind="ExternalOutput")

    with tile.TileContext(nc) as tc:
        _emit(nc, tc, locals())
    nc.compile()
    return nc


def _col1(dram_1d, lo, sz):
    return dram_1d[lo:lo + sz].rearrange("(c o) -> c o", o=1)


def _emit(nc, tc, io):
    from contextlib import ExitStack
    ctx = ExitStack()
    pers = ctx.enter_context(tc.tile_pool(name="pers", bufs=1))
    wp = ctx.enter_context(tc.tile_pool(name="wp", bufs=2))
    gt = ctx.enter_context(tc.tile_pool(name="gt", bufs=6))
    pp = ctx.enter_context(tc.tile_pool(name="pp", bufs=3))
    qt = ctx.enter_context(tc.tile_pool(name="qt", bufs=7))
    et = ctx.enter_context(tc.tile_pool(name="et", bufs=2))
    xc = ctx.enter_context(tc.tile_pool(name="xc", bufs=1))
    psp = ctx.enter_context(tc.tile_pool(name="psp", bufs=8, space="PSUM"))

    # ---- persistent planes (f32r, zero borders) ----
    def plane(name):
        return pers.tile([128, IPP, PD, PD], F32R, name=name)

    HE, HD = plane("HE"), plane("HD")
    S1, S2, S3 = plane("S1"), plane("S2"), plane("S3")
    D1, D2, D3 = plane("D1"), plane("D2"), plane("D3")
    IE = [plane(f"IE{i}") for i in range(4)]
    CE = pers.tile([128, NPIX], F32, name="CE")
    CD = pers.tile([128, NPIX], F32, name="CD")
    UT = pers.tile([128, 16, NPIX], F32R, name="UT")
    ZERO = pers.tile([128, 256], F32, name="ZERO")
    vt = pers.tile([7, IPP], F32, name="vt")
    VB = pers.tile([7, 256], F32, name="VB")
    KACC = pers.tile([64, 1], F32, name="KACC")

    for t in (HE, HD, S1, S2, S3, D1, D2, D3, *IE):
        nc.vector.memset(t[:].bitcast(F32), 0.0)
    nc.vector.memset(ZERO[:], 0.0)
    nc.vector.memset(KACC[:], 0.0)

    # ---- small persistent weights / biases ----
    w_rd_sb = pers.tile([3, 16, 3], F32R, name="w_rd_sb")
    nc.sync.dma_start(out=w_rd_sb[:],
                      in_=io["w_rd"].rearrange("t i o -> i t o").bitcast(F32R))
    w_ob_sb = pers.tile([128, 3], F32R, name="w_ob_sb")
    nc.sync.dma_start(out=w_ob_sb[:], in_=io["w_ob"][:].bitcast(F32R))

    def bias_tile(dram, lo, sz, name):
        t = pers.tile([sz, 1], F32, name=name)
        nc.sync.dma_start(out=t[:], in_=_col1(dram, lo, sz))
        return t

    pb = bias_tile(io["b_pri"], 0, 128, "pb")
    qb = bias_tile(io["b_pos"], 0, 128, "qb")
    ob = bias_tile(io["b_ob"], 0, 3, "ob")
    gbe = [bias_tile(io["b_eg"], o0, sz, f"gbe{i}")
           for i, (o0, sz) in enumerate(CO_G)]
    gbd = [bias_tile(io["b_dg"], o0, sz, f"gbd{i}")
           for i, (o0, sz) in enumerate(CO_G)]
    tbe = [bias_tile(io["b_et"], o0, sz, f"tbe{i}")
           for i, (o0, sz) in enumerate(CO_ET)]
    tbd = [bias_tile(io["b_dt"], o0, sz, f"tbd{i}")
           for i, (o0, sz) in enumerate(CO_DT)]

    # ---- conv-as-matmul block ----
    def conv(wdram, taps, ci_chunks, co_chunks, rhs_planes, consumer):
        """rhs_planes[k] -> plane tile for ci chunk k; consumer(j, psum)."""
        for j, (o0, osz) in enumerate(co_chunks):
            psum = psp.tile([osz, IPP, HS, HS], F32, tag="ps", name=f"ps_{o0}")
            n, last = 0, len(ci_chunks) * taps - 1
            for k, (i0, isz) in enumerate(ci_chunks):
                wt = wp.tile([isz, taps, osz], F32R, tag="w", name="wt")
                nc.sync.dma_start(
                    out=wt[:],
                    in_=wdram[:, i0:i0 + isz, o0:o0 + osz]
                        .rearrange("t i o -> i t o").bitcast(F32R))
                pl = rhs_planes[k]
                for tp in range(taps):
                    ky, kx = divmod(tp, 5)
                    rhs = pl[0:isz, :, ky:ky + HS, kx:kx + HS]
                    nc.tensor.matmul(psum[:], wt[:, tp, :], rhs,
                                     start=(n == 0), stop=(n == last))
                    n += 1
            consumer(j, psum)

    # =========================== pass loop ===========================
    for p in range(NPASS):
        # ---- pass setup ----
        for t in (HE, HD):
            nc.vector.memset(t[:].bitcast(F32), 0.0)
        nc.vector.memset(CE[:], 0.0)
        nc.vector.memset(CD[:], 0.0)

        for i in range(IPP):
            img = IPP * p + i
            # read conv: x (stride-4 4x4) -> xr in S1[0:3]
            XT = xc.tile([3, 64, 64], F32R, tag="xc", name="XT")
            nc.sync.dma_start(out=XT[:], in_=io["xin"][img].bitcast(F32R))
            XR = XT[:].rearrange("c (y a) (x b) -> c y a x b", a=4, b=4)
            psx = psp.tile([3, HS, HS], F32, tag="ps", name="psx")
            for tp in range(16):
                ky, kx = divmod(tp, 4)
                nc.tensor.matmul(psx[:], w_rd_sb[:, tp, :],
                                 XR[:, :, ky, :, kx],
                                 start=(tp == 0), stop=(tp == 15))
            nc.scalar.activation(out=S1[0:3, i, INT, INT], in_=psx[:],
                                 func=AF.Copy)
            # vb: v -> [7,1] tile, ACT-broadcast to [7,256], DMA-shift into
            # the (unaligned) plane partitions
            nc.sync.dma_start(out=vt[:, i:i + 1],
                              in_=io["vin"][img].rearrange("(c o) -> c o", o=1))
            nc.scalar.activation(out=VB[:], in_=ZERO[0:7, :],
                                 func=AF.Identity, bias=vt[:, i:i + 1])
            nc.sync.dma_start(out=S1[3:10, i, INT, INT],
                              in_=VB[:].bitcast(F32R))
            nc.sync.dma_start(out=D1[64:71, i, INT, INT],
                              in_=VB[:].bitcast(F32R))
            # rb chunks
            rb = io["rin"]
            for dst, dp0, c0, cn in (
                (S1, 10, 0, 118), (S2, 0, 118, 128), (S3, 0, 246, 10),
                (D1, 71, 0, 57), (D2, 0, 57, 128), (D3, 0, 185, 71),
            ):
                nc.sync.dma_start(
                    out=dst[dp0:dp0 + cn, i, INT, INT],
                    in_=rb[img, c0:c0 + cn].bitcast(F32R))

        # ---- step loop ----
        for t in range(NL):
            # prior conv on h_dec -> PP_p (mu 0:63, lv 64:127), PLV_p
            PP_p = pp.tile([128, NPIX], F32, tag="pp", name="PP_p")
            PLV_p = qt.tile([64, NPIX], F32, tag="qt", name="PLV_p")

            def prior_con(j, psum, _o=PP_p, _s=PLV_p):
                nc.scalar.activation(out=_o[:], in_=psum[:],
                                     func=AF.Identity, bias=pb[:])
                nc.gpsimd.dma_start(out=_s[:], in_=_o[64:128, :])

            conv(io["w_pri"], 25, CO_1, CO_1, [HD], prior_con)

            # enc transform + input assembly -> IE
            cat = [HD, S1, S2, S3]

            def et_con(j, psum):
                _, osz = CO_ET[j]
                for i in range(IPP):
                    nc.vector.scalar_tensor_tensor(
                        IE[j][0:osz, i, INT, INT], psum[0:osz, i],
                        tbe[j][:], cat[j][0:osz, i, INT, INT],
                        OP.add, OP.add)

            conv(io["w_et"], 25, CO_1, CO_ET, [HE], et_con)

            # enc gates + LSTM update
            _lstm(nc, gt, io["w_eg"], conv, ENC_CI, IE, gbe, CE, HE)

            # post conv on (new) h_enc -> PP_q, PLV_q
            PP_q = pp.tile([128, NPIX], F32, tag="pp", name="PP_q")
            PLV_q = qt.tile([64, NPIX], F32, tag="qt", name="PLV_q")

            def post_con(j, psum, _o=PP_q, _s=PLV_q):
                nc.scalar.activation(out=_o[:], in_=psum[:],
                                     func=AF.Identity, bias=qb[:])
                nc.gpsimd.dma_start(out=_s[:], in_=_o[64:128, :])

            conv(io["w_pos"], 25, CO_1, CO_1, [HE], post_con)

            # z = q_mu + exp(0.5 q_lv) * eps -> D1[0:64] interior
            ET = et.tile([64, NPIX], F32, tag="et", name="ET")
            for i in range(IPP):
                nc.gpsimd.dma_start(
                    out=ET[:, i * 256:(i + 1) * 256],
                    in_=io["epsin"][t, IPP * p + i]
                        .rearrange("c y x -> c (y x)"))
            E5 = qt.tile([64, NPIX], F32, tag="qt", name="E5")
            nc.scalar.activation(out=E5[:], in_=PLV_q[:], func=AF.Exp,
                                 scale=0.5)
            M = qt.tile([64, NPIX], F32, tag="qt", name="M")
            nc.vector.tensor_tensor(M[:], E5[:], ET[:], OP.mult)
            for i in range(IPP):
                nc.vector.tensor_tensor(
                    D1[0:64, i, INT, INT], M[:, i * 256:(i + 1) * 256],
                    PP_q[0:64, i * 256:(i + 1) * 256], OP.add)

            # kl element: exp(A) + (p_mu-q_mu)^2*exp(-p_lv) - 1 - A,
            # A = q_lv - p_lv ; accumulate row-sums into KACC
            A = qt.tile([64, NPIX], F32, tag="qt", name="A")
            nc.vector.tensor_tensor(A[:], PLV_q[:], PLV_p[:], OP.subtract)
            EA = qt.tile([64, NPIX], F32, tag="qt", name="EA")
            nc.scalar.activation(out=EA[:], in_=A[:], func=AF.Exp)
            NLt = qt.tile([64, NPIX], F32, tag="qt", name="NLt")
            nc.scalar.activation(out=NLt[:], in_=PLV_p[:], func=AF.Exp,
                                 scale=-1.0)
            Bd = qt.tile([64, NPIX], F32, tag="qt", name="Bd")
            nc.vector.tensor_tensor(Bd[:], PP_p[0:64, :], PP_q[0:64, :],
                                    OP.subtract)
            B2 = qt.tile([64, NPIX], F32, tag="qt", name="B2")
            nc.scalar.activation(out=B2[:], in_=Bd[:], func=AF.Square)
            Cc = qt.tile([64, NPIX], F32, tag="qt", name="Cc")
            nc.vector.tensor_tensor(Cc[:], B2[:], NLt[:], OP.mult)
            T1 = qt.tile([64, NPIX], F32, tag="qt", name="T1")
            nc.vector.tensor_tensor(T1[:], EA[:], Cc[:], OP.add)
            T2 = qt.tile([64, NPIX], F32, tag="qt", name="T2")
            nc.vector.scalar_tensor_tensor(T2[:], T1[:], -1.0, A[:],
                                           OP.add, OP.subtract)
            R = qt.tile([64, 1], F32, tag="qr", name="R")
            nc.vector.tensor_reduce(R[:], T2[:], mybir.AxisListType.X, OP.add)
            nc.vector.tensor_tensor(KACC[:], KACC[:], R[:], OP.add)

            # dec transform + input assembly -> ID (= IE[0:3] reused)
            dcat = [D1, D2, D3]

            def dt_con(j, psum):
                _, osz = CO_DT[j]
                for i in range(IPP):
                    nc.vector.scalar_tensor_tensor(
                        IE[j][0:osz, i, INT, INT], psum[0:osz, i],
                        tbd[j][:], dcat[j][0:osz, i, INT, INT],
                        OP.add, OP.add)

            conv(io["w_dt"], 25, CO_1, CO_DT, [HD], dt_con)

            # dec gates + LSTM update
            _lstm(nc, gt, io["w_dg"], conv, DEC_CI, IE, gbd, CD, HD)

            # write conv (4x4 stride-4 transpose conv) -> UT tap-major
            wt = wp.tile([128, 16, 128], F32R, tag="w", name="wt_wr")
            nc.sync.dma_start(
                out=wt[:],
                in_=io["w_wr"].rearrange("t i o -> i t o").bitcast(F32R))
            for tp in range(16):
                psw = psp.tile([128, NPIX], F32, tag="ps", name="psw")
                nc.tensor.matmul(psw[:], wt[:, tp, :],
                                 HD[:, :, INT, INT], start=True, stop=True)
                if t == 0:
                    nc.scalar.activation(out=UT[:, tp, :], in_=psw[:],
                                         func=AF.Copy)
                else:
                    nc.vector.tensor_tensor(UT[:, tp, :], UT[:, tp, :],
                                            psw[:], OP.add)

        # ---- pass end: obs 1x1 conv + sigmoid -> canvas ----
        CV = xc.tile([3, IPP, 64, 64], F32, tag="xc", name="CV")
        CVr = CV[:].rearrange("c i (y a) (x b) -> c i y a x b", a=4, b=4)
        for tp in range(16):
            pso = psp.tile([3, IPP, HS, HS], F32, tag="ps", name="pso")
            nc.tensor.matmul(pso[:], w_ob_sb[:], UT[:, tp, :],
                             start=True, stop=True)
            ky, kx = divmod(tp, 4)
            for i in range(IPP):
                nc.scalar.activation(out=CVr[:, i, :, ky, :, kx],
                                     in_=pso[:, i], func=AF.Sigmoid,
                                     bias=ob[:])
        for i in range(IPP):
            nc.sync.dma_start(out=io["canvas"][IPP * p + i], in_=CV[:, i])

    # ---- kl finalize: sum partitions, scale 0.5 ----
    KF = pers.tile([1, 1], F32, name="KF")
    nc.gpsimd.tensor_reduce(KF[:], KACC[:], mybir.AxisListType.C, OP.add)
    KS = pers.tile([1, 1], F32, name="KS")
    nc.scalar.activation(out=KS[:], in_=KF[:], func=AF.Copy, scale=0.5)
    nc.sync.dma_start(out=io["klout"][:], in_=KS[:])
    ctx.close()


def _lstm(nc, gt, wdram, conv, ci_chunks, inp_planes, gb, C, Hplane):
    """Gate convs (f,i,o,s co-chunks) + LSTM state update. Writes C and the
    interior of Hplane."""
    sig = [None] * 4

    def gate_con(j, psum):
        g = gt.tile([128, NPIX], F32, tag="gt", name=f"g{j}")
        fn = AF.Tanh if j == 3 else AF.Sigmoid
        nc.scalar.activation(out=g[:], in_=psum[:], func=fn, bias=gb[j][:])
        sig[j] = g

    conv(wdram, 25, ci_chunks, CO_G, inp_planes, gate_con)
    FS, IS, OS, ST = sig
    M1 = gt.tile([128, NPIX], F32, tag="gt", name="M1")
    nc.vector.tensor_tensor(M1[:], FS[:], C[:], OP.mult)
    M2 = gt.tile([128, NPIX], F32, tag="gt", name="M2")
    nc.vector.tensor_tensor(M2[:], IS[:], ST[:], OP.mult)
    nc.vector.tensor_tensor(C[:], M1[:], M2[:], OP.add)
    TC = gt.tile([128, NPIX], F32, tag="gt", name="TC")
    nc.scalar.activation(out=TC[:], in_=C[:], func=AF.Tanh)
    for i in range(IPP):
        nc.vector.tensor_tensor(
            Hplane[:, i, INT, INT], OS[:, i * 256:(i + 1) * 256],
            TC[:, i * 256:(i + 1) * 256], OP.mult)


# ============================ host side ============================

_NC = None


def _get_nc():
    global _NC
    if _NC is None:
        _NC = _build()
    return _NC


def _prep_weights(inputs):
    def tap_major(w):
        # OIHW -> [kh*kw, I, O]
        kh, kw = w.shape[2], w.shape[3]
        return np.ascontiguousarray(
            w.transpose(2, 3, 1, 0).reshape(kh * kw, w.shape[1], w.shape[0])
        ).astype(np.float32)

    shared = {
        "w_pri": tap_major(inputs["prior_w"]),
        "w_pos": tap_major(inputs["post_w"]),
        "w_et": tap_major(inputs["enc_tw"]),
        "w_eg": tap_major(inputs["enc_gw"]),
        "w_dt": tap_major(inputs["dec_tw"]),
        "w_dg": tap_major(inputs["dec_gw"]),
        # write conv: out[o, 4y+ky, 4x+kx] += w[o, i, ky, kx] * h[i, y, x]
        # (verified vs lax.conv_transpose(transpose_kernel=True)) -> same
        # tap-major [tap, i, o] layout as the OIHW convs
        "w_wr": tap_major(inputs["write_w"]),
        "w_rd": tap_major(inputs["read_w"]),
        "w_ob": np.ascontiguousarray(
            inputs["obs_w"][:, :, 0, 0].T).astype(np.float32),
        "b_pri": np.asarray(inputs["prior_b"], np.float32),
        "b_pos": np.asarray(inputs["post_b"], np.float32),
        "b_eg": np.asarray(inputs["enc_gb"], np.float32),
        "b_et": np.asarray(inputs["enc_tb"], np.float32),
        "b_dg": np.asarray(inputs["dec_gb"], np.float32),
        "b_dt": np.asarray(inputs["dec_tb"], np.float32),
        "b_ob": np.asarray(inputs["obs_b"], np.float32),
    }
    return shared


def make_in_maps(inputs):
    shared = _prep_weights(inputs)
    x = np.asarray(inputs["x"], np.float32)
    v = np.asarray(inputs["v"], np.float32)
    r = np.asarray(inputs["r"], np.float32)
    eps = np.asarray(inputs["eps"], np.float32)
    maps = []
    for c in range(NCORES):
        s = slice(c * BPC, (c + 1) * BPC)
        m = dict(shared)
        m["xin"] = np.ascontiguousarray(x[s])
        m["vin"] = np.ascontiguousarray(v[s])
        m["rin"] = np.ascontiguousarray(r[s])
        m["epsin"] = np.ascontiguousarray(eps[:, s])
        maps.append(m)
    return maps


def kernel(**inputs):
    nc = _get_nc()
    in_maps = make_in_maps(inputs)
    res = run_bass_kernel_spmd(nc, in_maps, core_ids=list(range(NCORES)))
    canvas = np.concatenate([res.results[c]["canvas"] for c in range(NCORES)],
                            axis=0)
    kl = sum(float(res.results[c]["klout"][0, 0]) for c in range(NCORES)) / B
    return canvas, np.float32(kl)


# revision 8
# speedup vs baseline: 1.6585x; 1.6585x over previous
"""ConvolutionalDRAW Trainium2 kernel (Bass/Tile, 8 NeuronCores, pure data parallel).

Strategy
--------
- Pure data parallel: 8 images per core (batch 64 / 8 cores). No collectives;
  the kl partial sums are combined on the host during unsharding.
- Each core processes its 8 images in 4 passes of 2 images. Per pass the full
  8-step DRAW recurrence runs with all activations SBUF-resident.
- Convolutions are computed channel-major as shift-and-accumulate matmuls:
  activations live in zero-padded planes [ch, img, 20, 20]; kernel tap (ky,kx)
  contributes one matmul lhsT=W[ci,co], rhs=plane[ci, :, ky:ky+16, kx:kx+16],
  accumulated in PSUM over (ci-chunk, tap). N = 2 img * 256 pix = 512.
- All matmuls run in float32r (full-rate fp32 on trn2 PE, ~19-bit mantissa).
- Weights are streamed from HBM per (conv, ci-chunk, co-chunk) block in a
  host-pretransposed [tap, ci, co] layout, double-buffered.
- The write transpose-conv (stride 4 = kernel 4) is 16 disjoint-tap matmuls
  accumulated into a tap-major u buffer; the 1x1 obs conv + sigmoid runs at
  pass end, assembled into canvas layout on-chip, then DMA'd out contiguously.
"""
import os
import numpy as np

import concourse.bass as bass
import concourse.mybir as mybir
import concourse.tile as tile
from concourse import bacc
from concourse.bass_utils import run_bass_kernel_spmd

F32 = mybir.dt.float32
F32R = mybir.dt.float32r
AF = mybir.ActivationFunctionType
OP = mybir.AluOpType

NCORES = 8
B = 64
BPC = B // NCORES          # images per core
IPP = 2                    # images per pass
NPASS = BPC // IPP
NL = 8                     # DRAW steps
HS, PD = 16, 20            # spatial, padded
NPIX = IPP * HS * HS       # matmul free size (512)
INT = slice(2, 18)         # interior of padded plane

# channel chunking (ci side) of the two recurrent conv inputs
ENC_CI = [(0, 128), (128, 128), (256, 128), (384, 10)]    # enc concat: 394
DEC_CI = [(0, 128), (128, 128), (256, 71)]                # dec concat: 327
CO_G = [(0, 128), (128, 128), (256, 128), (384, 128)]     # gates: 512
CO_ET = [(0, 128), (128, 128), (256, 128), (384, 10)]     # enc transform out: 394
CO_DT = [(0, 128), (128, 128), (256, 71)]                 # dec transform out: 327
CO_1 = [(0, 128)]


def _build():
    nc = bacc.Bacc("TRN2", target_bir_lowering=False, debug=False,
                   num_devices=NCORES)

    # ---- DRAM I/O (per core) ----
    xin = nc.dram_tensor("xin", [BPC, 3, 64, 64], F32, kind="ExternalInput")
    vin = nc.dram_tensor("vin", [BPC, 7], F32, kind="ExternalInput")
    rin = nc.dram_tensor("rin", [BPC, 256, HS, HS], F32, kind="ExternalInput")
    epsin = nc.dram_tensor("epsin", [NL, BPC, 64, HS, HS], F32,
                           kind="ExternalInput")
    # host-pretransposed weights [tap, ci, co]
    w_pri = nc.dram_tensor("w_pri", [25, 128, 128], F32, kind="ExternalInput")
    w_pos = nc.dram_tensor("w_pos", [25, 128, 128], F32, kind="ExternalInput")
    w_et = nc.dram_tensor("w_et", [25, 128, 394], F32, kind="ExternalInput")
    w_eg = nc.dram_tensor("w_eg", [25, 394, 512], F32, kind="ExternalInput")
    w_dt = nc.dram_tensor("w_dt", [25, 128, 327], F32, kind="ExternalInput")
    w_dg = nc.dram_tensor("w_dg", [25, 327, 512], F32, kind="ExternalInput")
    w_wr = nc.dram_tensor("w_wr", [16, 128, 128], F32, kind="ExternalInput")
    w_rd = nc.dram_tensor("w_rd", [16, 3, 3], F32, kind="ExternalInput")
    w_ob = nc.dram_tensor("w_ob", [128, 3], F32, kind="ExternalInput")
    b_pri = nc.dram_tensor("b_pri", [128], F32, kind="ExternalInput")
    b_pos = nc.dram_tensor("b_pos", [128], F32, kind="ExternalInput")
    b_eg = nc.dram_tensor("b_eg", [512], F32, kind="ExternalInput")
    b_et = nc.dram_tensor("b_et", [394], F32, kind="ExternalInput")
    b_dg = nc.dram_tensor("b_dg", [512], F32, kind="ExternalInput")
    b_dt = nc.dram_tensor("b_dt", [327], F32, kind="ExternalInput")
    b_ob = nc.dram_tensor("b_ob", [3], F32, kind="ExternalInput")

    canvas = nc.dram_tensor("canvas", [BPC, 3, 64, 64], F32,
                            kind="ExternalOutput")
    klout = nc.dram_tensor("klout", [1, 1], F32, kind="ExternalOutput")

    with tile.TileContext(nc) as tc:
        _emit(nc, tc, locals())
    nc.compile()
    return nc


def _col1(dram_1d, lo, sz):
    return dram_1d[lo:lo + sz].rearrange("(c o) -> c o", o=1)


def _emit(nc, tc, io):
    from contextlib import ExitStack
    ctx = ExitStack()
    pers = ctx.enter_context(tc.tile_pool(name="pers", bufs=1))
    wp = ctx.enter_context(tc.tile_pool(name="wp", bufs=2))
    gt = ctx.enter_context(tc.tile_pool(name="gt", bufs=6))
    pp = ctx.enter_context(tc.tile_pool(name="pp", bufs=3))
    qt = ctx.enter_context(tc.tile_pool(name="qt", bufs=7))
    et = ctx.enter_context(tc.tile_pool(name="et", bufs=2))
    xc = ctx.enter_context(tc.tile_pool(name="xc", bufs=1))
    psp = ctx.enter_context(tc.tile_pool(name="psp", bufs=8, space="PSUM"))

    # ---- persistent planes (f32r, zero borders) ----
    def plane(name):
        return pers.tile([128, IPP, PD, PD], F32R, name=name)

    HE, HD = plane("HE"), plane("HD")
    S1, S2, S3 = plane("S1"), plane("S2"), plane("S3")
    D1, D2, D3 = plane("D1"), plane("D2"), plane("D3")
    IE = [plane(f"IE{i}") for i in range(4)]
    CE = pers.tile([128, NPIX], F32, name="CE")
    CD = pers.tile([128, NPIX], F32, name="CD")
    UT = pers.tile([128, 16, NPIX], F32R, name="UT")
    ZERO = pers.tile([128, 256], F32, name="ZERO")
    vt = pers.tile([7, IPP], F32, name="vt")
    VB = pers.tile([7, 256], F32, name="VB")
    KACC = pers.tile([64, 1], F32, name="KACC")

    for t in (HE, HD, S1, S2, S3, D1, D2, D3, *IE):
        nc.vector.memset(t[:].bitcast(F32), 0.0)
    nc.vector.memset(ZERO[:], 0.0)
    nc.vector.memset(KACC[:], 0.0)

    # ---- small persistent weights / biases ----
    w_rd_sb = pers.tile([3, 16, 3], F32R, name="w_rd_sb")
    nc.sync.dma_start(out=w_rd_sb[:],
                      in_=io["w_rd"].rearrange("t i o -> i t o").bitcast(F32R))
    w_ob_sb = pers.tile([128, 3], F32R, name="w_ob_sb")
    nc.sync.dma_start(out=w_ob_sb[:], in_=io["w_ob"][:].bitcast(F32R))

    def bias_tile(dram, lo, sz, name):
        t = pers.tile([sz, 1], F32, name=name)
        nc.sync.dma_start(out=t[:], in_=_col1(dram, lo, sz))
        return t

    pb = bias_tile(io["b_pri"], 0, 128, "pb")
    qb = bias_tile(io["b_pos"], 0, 128, "qb")
    ob = bias_tile(io["b_ob"], 0, 3, "ob")
    gbe = [bias_tile(io["b_eg"], o0, sz, f"gbe{i}")
           for i, (o0, sz) in enumerate(CO_G)]
    gbd = [bias_tile(io["b_dg"], o0, sz, f"gbd{i}")
           for i, (o0, sz) in enumerate(CO_G)]
    tbe = [bias_tile(io["b_et"], o0, sz, f"tbe{i}")
           for i, (o0, sz) in enumerate(CO_ET)]
    tbd = [bias_tile(io["b_dt"], o0, sz, f"tbd{i}")
           for i, (o0, sz) in enumerate(CO_DT)]

    # ---- conv-as-matmul block ----
    lite_w = bool(int(os.environ.get("DRAW_LITE_W", "0")))

    def conv(wdram, taps, ci_chunks, co_chunks, rhs_planes, consumer):
        """rhs_planes[k] -> plane tile for ci chunk k; consumer(j, psum)."""
        for j, (o0, osz) in enumerate(co_chunks):
            psum = psp.tile([osz, IPP, HS, HS], F32, tag="ps", name=f"ps_{o0}")
            n, last = 0, len(ci_chunks) * taps - 1
            for k, (i0, isz) in enumerate(ci_chunks):
                wt = wp.tile([isz, taps, osz], F32R, tag="w", name="wt")
                if lite_w:
                    nc.sync.dma_start(
                        out=wt[:, 0:1, :],
                        in_=wdram[0:1, i0:i0 + isz, o0:o0 + osz]
                            .rearrange("t i o -> i t o").bitcast(F32R))
                else:
                    nc.sync.dma_start(
                        out=wt[:],
                        in_=wdram[:, i0:i0 + isz, o0:o0 + osz]
                            .rearrange("t i o -> i t o").bitcast(F32R))
                pl = rhs_planes[k]
                for tp in range(taps):
                    ky, kx = divmod(tp, 5)
                    rhs = pl[0:isz, :, ky:ky + HS, kx:kx + HS]
                    nc.tensor.matmul(psum[:],
                                     wt[:, 0 if lite_w else tp, :], rhs,
                                     start=(n == 0), stop=(n == last))
                    n += 1
            consumer(j, psum)

    # =========================== pass loop ===========================
    for p in range(NPASS):
        # ---- pass setup ----
        for t in (HE, HD):
            nc.vector.memset(t[:].bitcast(F32), 0.0)
        nc.vector.memset(CE[:], 0.0)
        nc.vector.memset(CD[:], 0.0)

        for i in range(IPP):
            img = IPP * p + i
            # read conv: x (stride-4 4x4) -> xr in S1[0:3]
            XT = xc.tile([3, 64, 64], F32R, tag="xc", name="XT")
            nc.sync.dma_start(out=XT[:], in_=io["xin"][img].bitcast(F32R))
            XR = XT[:].rearrange("c (y a) (x b) -> c y a x b", a=4, b=4)
            psx = psp.tile([3, HS, HS], F32, tag="ps", name="psx")
            for tp in range(16):
                ky, kx = divmod(tp, 4)
                nc.tensor.matmul(psx[:], w_rd_sb[:, tp, :],
                                 XR[:, :, ky, :, kx],
                                 start=(tp == 0), stop=(tp == 15))
            nc.scalar.activation(out=S1[0:3, i, INT, INT], in_=psx[:],
                                 func=AF.Copy)
            # vb: v -> [7,1] tile, ACT-broadcast to [7,256], DMA-shift into
            # the (unaligned) plane partitions
            nc.sync.dma_start(out=vt[:, i:i + 1],
                              in_=io["vin"][img].rearrange("(c o) -> c o", o=1))
            nc.scalar.activation(out=VB[:], in_=ZERO[0:7, :],
                                 func=AF.Identity, bias=vt[:, i:i + 1])
            nc.sync.dma_start(out=S1[3:10, i, INT, INT],
                              in_=VB[:].bitcast(F32R))
            nc.sync.dma_start(out=D1[64:71, i, INT, INT],
                              in_=VB[:].bitcast(F32R))
            # rb chunks
            rb = io["rin"]
            for dst, dp0, c0, cn in (
                (S1, 10, 0, 118), (S2, 0, 118, 128), (S3, 0, 246, 10),
                (D1, 71, 0, 57), (D2, 0, 57, 128), (D3, 0, 185, 71),
            ):
                nc.sync.dma_start(
                    out=dst[dp0:dp0 + cn, i, INT, INT],
                    in_=rb[img, c0:c0 + cn].bitcast(F32R))

        # ---- step loop ----
        for t in range(NL):
            # prior conv on h_dec -> PP_p (mu 0:63, lv 64:127), PLV_p
            PP_p = pp.tile([128, NPIX], F32, tag="pp", name="PP_p")
            PLV_p = qt.tile([64, NPIX], F32, tag="qt", name="PLV_p")

            def prior_con(j, psum, _o=PP_p, _s=PLV_p):
                nc.scalar.activation(out=_o[:], in_=psum[:],
                                     func=AF.Identity, bias=pb[:])
                nc.gpsimd.dma_start(out=_s[:], in_=_o[64:128, :])

            conv(io["w_pri"], 25, CO_1, CO_1, [HD], prior_con)

            # enc transform + input assembly -> IE
            cat = [HD, S1, S2, S3]

            def et_con(j, psum):
                _, osz = CO_ET[j]
                for i in range(IPP):
                    nc.vector.scalar_tensor_tensor(
                        IE[j][0:osz, i, INT, INT], psum[0:osz, i],
                        tbe[j][:], cat[j][0:osz, i, INT, INT],
                        OP.add, OP.add)

            conv(io["w_et"], 25, CO_1, CO_ET, [HE], et_con)

            # enc gates + LSTM update
            _lstm(nc, gt, io["w_eg"], conv, ENC_CI, IE, gbe, CE, HE)

            # post conv on (new) h_enc -> PP_q, PLV_q
            PP_q = pp.tile([128, NPIX], F32, tag="pp", name="PP_q")
            PLV_q = qt.tile([64, NPIX], F32, tag="qt", name="PLV_q")

            def post_con(j, psum, _o=PP_q, _s=PLV_q):
                nc.scalar.activation(out=_o[:], in_=psum[:],
                                     func=AF.Identity, bias=qb[:])
                nc.gpsimd.dma_start(out=_s[:], in_=_o[64:128, :])

            conv(io["w_pos"], 25, CO_1, CO_1, [HE], post_con)

            # z = q_mu + exp(0.5 q_lv) * eps -> D1[0:64] interior
            ET = et.tile([64, NPIX], F32, tag="et", name="ET")
            for i in range(IPP):
                nc.gpsimd.dma_start(
                    out=ET[:, i * 256:(i + 1) * 256],
                    in_=io["epsin"][t, IPP * p + i]
                        .rearrange("c y x -> c (y x)"))
            E5 = qt.tile([64, NPIX], F32, tag="qt", name="E5")
            nc.scalar.activation(out=E5[:], in_=PLV_q[:], func=AF.Exp,
                                 scale=0.5)
            M = qt.tile([64, NPIX], F32, tag="qt", name="M")
            nc.vector.tensor_tensor(M[:], E5[:], ET[:], OP.mult)
            for i in range(IPP):
                nc.vector.tensor_tensor(
                    D1[0:64, i, INT, INT], M[:, i * 256:(i + 1) * 256],
                    PP_q[0:64, i * 256:(i + 1) * 256], OP.add)

            # kl element: exp(A) + (p_mu-q_mu)^2*exp(-p_lv) - 1 - A,
            # A = q_lv - p_lv ; accumulate row-sums into KACC
            A = qt.tile([64, NPIX], F32, tag="qt", name="A")
            nc.vector.tensor_tensor(A[:], PLV_q[:], PLV_p[:], OP.subtract)
            EA = qt.tile([64, NPIX], F32, tag="qt", name="EA")
            nc.scalar.activation(out=EA[:], in_=A[:], func=AF.Exp)
            NLt = qt.tile([64, NPIX], F32, tag="qt", name="NLt")
            nc.scalar.activation(out=NLt[:], in_=PLV_p[:], func=AF.Exp,
                                 scale=-1.0)
            Bd = qt.tile([64, NPIX], F32, tag="qt", name="Bd")
            nc.vector.tensor_tensor(Bd[:], PP_p[0:64, :], PP_q[0:64, :],
                                    OP.subtract)
            B2 = qt.tile([64, NPIX], F32, tag="qt", name="B2")
            nc.scalar.activation(out=B2[:], in_=Bd[:], func=AF.Square)
            Cc = qt.tile([64, NPIX], F32, tag="qt", name="Cc")
            nc.vector.tensor_tensor(Cc[:], B2[:], NLt[:], OP.mult)
            T1 = qt.tile([64, NPIX], F32, tag="qt", name="T1")
            nc.vector.tensor_tensor(T1[:], EA[:], Cc[:], OP.add)
            T2 = qt.tile([64, NPIX], F32, tag="qt", name="T2")
            nc.vector.scalar_tensor_tensor(T2[:], T1[:], -1.0, A[:],
                                           OP.add, OP.subtract)
            R = qt.tile([64, 1], F32, tag="qr", name="R")
            nc.vector.tensor_reduce(R[:], T2[:], mybir.AxisListType.X, OP.add)
            nc.vector.tensor_tensor(KACC[:], KACC[:], R[:], OP.add)

            # dec transform + input assembly -> ID (= IE[0:3] reused)
            dcat = [D1, D2, D3]

            def dt_con(j, psum):
                _, osz = CO_DT[j]
                for i in range(IPP):
                    nc.vector.scalar_tensor_tensor(
                        IE[j][0:osz, i, INT, INT], psum[0:osz, i],
                        tbd[j][:], dcat[j][0:osz, i, INT, INT],
                        OP.add, OP.add)

            conv(io["w_dt"], 25, CO_1, CO_DT, [HD], dt_con)

            # dec gates + LSTM update
            _lstm(nc, gt, io["w_dg"], conv, DEC_CI, IE, gbd, CD, HD)

            # write conv (4x4 stride-4 transpose conv) -> UT tap-major
            wt = wp.tile([128, 16, 128], F32R, tag="w", name="wt_wr")
            nc.sync.dma_start(
                out=wt[:],
                in_=io["w_wr"].rearrange("t i o -> i t o").bitcast(F32R))
            for tp in range(16):
                psw = psp.tile([128, NPIX], F32, tag="ps", name="psw")
                nc.tensor.matmul(psw[:], wt[:, tp, :],
                                 HD[:, :, INT, INT], start=True, stop=True)
                if t == 0:
                    nc.scalar.activation(out=UT[:, tp, :], in_=psw[:],
                                         func=AF.Copy)
                else:
                    nc.vector.tensor_tensor(UT[:, tp, :], UT[:, tp, :],
                                            psw[:], OP.add)

        # ---- pass end: obs 1x1 conv + sigmoid -> canvas ----
        CV = xc.tile([3, IPP, 64, 64], F32, tag="xc", name="CV")
        CVr = CV[:].rearrange("c i (y a) (x b) -> c i y a x b", a=4, b=4)
        for tp in range(16):
            pso = psp.tile([3, IPP, HS, HS], F32, tag="ps", name="pso")
            nc.tensor.matmul(pso[:], w_ob_sb[:], UT[:, tp, :],
                             start=True, stop=True)
            ky, kx = divmod(tp, 4)
            for i in range(IPP):
                nc.scalar.activation(out=CVr[:, i, :, ky, :, kx],
                                     in_=pso[:, i], func=AF.Sigmoid,
                                     bias=ob[:])
        for i in range(IPP):
            nc.sync.dma_start(out=io["canvas"][IPP * p + i], in_=CV[:, i])

    # ---- kl finalize: sum partitions, scale 0.5 ----
    KF = pers.tile([1, 1], F32, name="KF")
    nc.gpsimd.tensor_reduce(KF[:], KACC[:], mybir.AxisListType.C, OP.add)
    KS = pers.tile([1, 1], F32, name="KS")
    nc.scalar.activation(out=KS[:], in_=KF[:], func=AF.Copy, scale=0.5)
    nc.sync.dma_start(out=io["klout"][:], in_=KS[:])
    ctx.close()


def _lstm(nc, gt, wdram, conv, ci_chunks, inp_planes, gb, C, Hplane):
    """Gate convs (f,i,o,s co-chunks) + LSTM state update. Writes C and the
    interior of Hplane."""
    sig = [None] * 4

    def gate_con(j, psum):
        g = gt.tile([128, NPIX], F32, tag="gt", name=f"g{j}")
        fn = AF.Tanh if j == 3 else AF.Sigmoid
        nc.scalar.activation(out=g[:], in_=psum[:], func=fn, bias=gb[j][:])
        sig[j] = g

    conv(wdram, 25, ci_chunks, CO_G, inp_planes, gate_con)
    FS, IS, OS, ST = sig
    M1 = gt.tile([128, NPIX], F32, tag="gt", name="M1")
    nc.vector.tensor_tensor(M1[:], FS[:], C[:], OP.mult)
    M2 = gt.tile([128, NPIX], F32, tag="gt", name="M2")
    nc.vector.tensor_tensor(M2[:], IS[:], ST[:], OP.mult)
    nc.vector.tensor_tensor(C[:], M1[:], M2[:], OP.add)
    TC = gt.tile([128, NPIX], F32, tag="gt", name="TC")
    nc.scalar.activation(out=TC[:], in_=C[:], func=AF.Tanh)
    for i in range(IPP):
        nc.vector.tensor_tensor(
            Hplane[:, i, INT, INT], OS[:, i * 256:(i + 1) * 256],
            TC[:, i * 256:(i + 1) * 256], OP.mult)


# ============================ host side ============================

_NC = None


def _get_nc():
    global _NC
    if _NC is None:
        _NC = _build()
    return _NC


def _prep_weights(inputs):
    def tap_major(w):
        # OIHW -> [kh*kw, I, O]
        kh, kw = w.shape[2], w.shape[3]
        return np.ascontiguousarray(
            w.transpose(2, 3, 1, 0).reshape(kh * kw, w.shape[1], w.shape[0])
        ).astype(np.float32)

    shared = {
        "w_pri": tap_major(inputs["prior_w"]),
        "w_pos": tap_major(inputs["post_w"]),
        "w_et": tap_major(inputs["enc_tw"]),
        "w_eg": tap_major(inputs["enc_gw"]),
        "w_dt": tap_major(inputs["dec_tw"]),
        "w_dg": tap_major(inputs["dec_gw"]),
        # write conv: out[o, 4y+ky, 4x+kx] += w[o, i, ky, kx] * h[i, y, x]
        # (verified vs lax.conv_transpose(transpose_kernel=True)) -> same
        # tap-major [tap, i, o] layout as the OIHW convs
        "w_wr": tap_major(inputs["write_w"]),
        "w_rd": tap_major(inputs["read_w"]),
        "w_ob": np.ascontiguousarray(
            inputs["obs_w"][:, :, 0, 0].T).astype(np.float32),
        "b_pri": np.asarray(inputs["prior_b"], np.float32),
        "b_pos": np.asarray(inputs["post_b"], np.float32),
        "b_eg": np.asarray(inputs["enc_gb"], np.float32),
        "b_et": np.asarray(inputs["enc_tb"], np.float32),
        "b_dg": np.asarray(inputs["dec_gb"], np.float32),
        "b_dt": np.asarray(inputs["dec_tb"], np.float32),
        "b_ob": np.asarray(inputs["obs_b"], np.float32),
    }
    return shared


def make_in_maps(inputs):
    shared = _prep_weights(inputs)
    x = np.asarray(inputs["x"], np.float32)
    v = np.asarray(inputs["v"], np.float32)
    r = np.asarray(inputs["r"], np.float32)
    eps = np.asarray(inputs["eps"], np.float32)
    maps = []
    for c in range(NCORES):
        s = slice(c * BPC, (c + 1) * BPC)
        m = dict(shared)
        m["xin"] = np.ascontiguousarray(x[s])
        m["vin"] = np.ascontiguousarray(v[s])
        m["rin"] = np.ascontiguousarray(r[s])
        m["epsin"] = np.ascontiguousarray(eps[:, s])
        maps.append(m)
    return maps


def kernel(**inputs):
    nc = _get_nc()
    in_maps = make_in_maps(inputs)
    res = run_bass_kernel_spmd(nc, in_maps, core_ids=list(range(NCORES)))
    canvas = np.concatenate([res.results[c]["canvas"] for c in range(NCORES)],
                            axis=0)
    kl = sum(float(res.results[c]["klout"][0, 0]) for c in range(NCORES)) / B
    return canvas, np.float32(kl)
